# revision 8
# baseline (speedup 1.0000x reference)
"""Trainium2 Bass kernel for nn_Decoder_3461743640648 (gnn_message_passing).

acc = bn_linear(f0)[idx0] + bn_linear(f1)[idx1] + bn_linear(f2)[idx2]
      + bn_linear(f3)                                  -> [240000, 256]

Strategy (8 cores, data-parallel over the 240000 fine points):
 - Training-mode BN folded into the linear: proj_l = f_l @ (W_l*scale_l)^T
   + b'_l with scale_l = gamma_l*rsqrt(var_l+eps) and
   b'_l = ((beta_l/gamma_l)*std_l - mean_l) @ (W_l*scale_l)^T. All four b'_l
   are summed into one b_all added once via a ones-column in the f3 operand.
 - proj0 [3750,256] / proj1 [15000,256] tables built redundantly on every
   core (from host-pre-transposed chunk layouts; streamed in row blocks,
   one stats pass + one matmul pass), stored to local DRAM, then
   row-gathered at fine resolution with dma_gather (int16 indices,
   3072 rows/call, multi-packet, alternating SWDGE queues).
 - Level 2 gathers RAW f2 rows as pairs: f2 viewed [30000, 256] so idx2>>1
   fits int16; a DVE predicated-copy selects the odd row where parity=1.
   The 128->256 projection happens on the PE at fine resolution.
 - f0/f1 stats: free-axis reduce + ACT Square(accum_out) on transposed
   blocks. f2 stats from a host-transposed per-core row-shard. f3 stats
   from one accumulated Gram matmul (the ones column makes Gram[:,64] the
   per-channel sums; the diagonal is sumsq). f2/f3 partial stats cross
   cores via one 2KB AllReduce, overlapped with the proj table builds.
 - Per-core point layout is column-major within gather calls:
   j = g*3072 + m*128 + p  ->  out[p, g*24+m, :].
"""
import sys

sys.path.insert(0, "/opt/trn_rl_repo")

import numpy as np  # noqa: E402

EPS = 1e-5
N_CORES = 8
NF = 240000          # fine points
NS = NF // N_CORES   # 30000 per core
NIDX = 2048          # rows per dma_gather call
NG = 15              # gather calls (groups) per core
NPAD = NG * NIDX     # 30720
MCOL = NIDX // 128   # 24 sub-tile columns per group
NCOL = NG * MCOL     # 240 output columns
N0, N1, N2, N3 = 3750, 15000, 60000, 240000
DOUT = 256

_prog_cache = None
_last_in_maps = None


def _build_program():
    from concourse import bass, bacc, mybir, tile
    from concourse.library_config import mlp
    from concourse.masks import make_identity

    f32 = mybir.dt.float32
    i16 = mybir.dt.int16
    ADD = mybir.AluOpType.add
    MUL = mybir.AluOpType.mult
    SUB = mybir.AluOpType.subtract

    nc = bacc.Bacc("TRN2", target_bir_lowering=False, debug=False,
                   num_devices=N_CORES, num_swdge_queues=2)

    def din(name, shape, dt=f32):
        return nc.dram_tensor(name, shape, dt, kind="ExternalInput").ap()

    # replicated inputs
    f0tc_d = din("f0tc", [128, 4, N0])        # f0.T chunk layout
    f1tc_d = din("f1tc", [128, 2, N1])        # f1.T chunk layout
    f2pair_d = din("f2pair", [N2 // 2, 256])  # f2 viewed as row pairs
    w0_d = din("w0tc", [128, 4, DOUT])
    w1_d = din("w1tc", [128, 2, DOUT])
    w2_d = din("w2t", [128, DOUT])
    w3_d = din("w3te", [65, DOUT])            # W3.T + zeros row
    g0_d = din("g0c", [128, 4]); bg0_d = din("bg0c", [128, 4])
    g1_d = din("g1c", [128, 2]); bg1_d = din("bg1c", [128, 2])
    g2_d = din("g2c", [128, 1]); bg2_d = din("bg2c", [128, 1])
    g3_d = din("g3ce", [65, 1]); bg3_d = din("bg3ce", [65, 1])
    # per-core inputs
    f2ts_d = din("f2ts", [128, N2 // 8])      # transposed f2 row-shard
    f3s_d = din("f3s", [128, NCOL, 65])       # f3 shard + ones col, grid layout
    ix0_d = din("idx0w", [128, NG, NIDX // 16], i16)
    ix1_d = din("idx1w", [128, NG, NIDX // 16], i16)
    ix2_d = din("idx2w", [128, NG, NIDX // 16], i16)
    par_d = din("par2", [128, NCOL, 1], mybir.dt.int8)
    out_d = nc.dram_tensor("out", [128, NCOL, DOUT], f32,
                           kind="ExternalOutput").ap()
    # internal DRAM
    proj0_d = nc.dram_tensor("proj0", [N0, DOUT], f32).ap()
    proj1_d = nc.dram_tensor("proj1", [N1, DOUT], f32).ap()
    cc_in_d = nc.dram_tensor("cc_in", [128, 4], f32).ap()
    cc_out_d = nc.dram_tensor("cc_out", [128, 4], f32,
                              addr_space="Shared").ap()

    NV = [NIDX] * (NG - 1) + [NS - (NG - 1) * NIDX]  # valid idx per call

    with tile.TileContext(nc) as tc:
        with tc.tile_pool(name="const", bufs=1) as cp:
            nc.gpsimd.load_library(mlp)
            ident = cp.tile([128, 128], f32)
            make_identity(nc, ident[:])
            eps_t = cp.tile([128, 1], f32)
            nc.vector.memset(eps_t[:], EPS)

            ix_t = []
            for ix_d in (ix0_d, ix1_d, ix2_d):
                t = cp.tile([128, NG, NIDX // 16], i16, tag=f"ix{len(ix_t)}")
                nc.sync.dma_start(out=t[:], in_=ix_d[...])
                ix_t.append(t)
            par_t = cp.tile([128, NCOL, 1], mybir.dt.int8)
            nc.sync.dma_start(out=par_t[:], in_=par_d[...])

            w2_t = cp.tile([128, DOUT], f32)
            w3_t = cp.tile([65, DOUT], f32)
            nc.sync.dma_start(out=w2_t[:], in_=w2_d[...])
            nc.sync.dma_start(out=w3_t[:], in_=w3_d[...])

            def fold(sum_ap, ssq_ap, g_ap, bg_ap, inv_n, P, K):
                """-> (scale [P,K], v [P,K]); v @ (W*scale)^T = bias b'."""
                mean = cp.tile([P, K], f32, tag="fold_mean")
                var = cp.tile([P, K], f32, tag="fold_var")
                std = cp.tile([P, K], f32, tag="fold_std")
                rstd = cp.tile([P, K], f32, tag="fold_rstd")
                sc = cp.tile([P, K], f32, tag="fold_sc")
                v = cp.tile([P, K], f32, tag="fold_v")
                nc.vector.tensor_scalar_mul(mean[:], sum_ap, inv_n)
                nc.vector.tensor_scalar_mul(var[:], ssq_ap, inv_n)
                nc.vector.tensor_tensor(out=std[:], in0=mean[:], in1=mean[:],
                                        op=MUL)
                nc.vector.tensor_tensor(out=var[:], in0=var[:], in1=std[:],
                                        op=SUB)
                nc.scalar.activation(
                    out=std[:], in_=var[:],
                    func=mybir.ActivationFunctionType.Sqrt,
                    bias=eps_t[:P, :])
                nc.vector.reciprocal(out=rstd[:], in_=std[:])
                nc.vector.tensor_tensor(out=sc[:], in0=rstd[:], in1=g_ap,
                                        op=MUL)
                nc.vector.tensor_tensor(out=v[:], in0=std[:], in1=bg_ap,
                                        op=MUL)
                nc.vector.tensor_tensor(out=v[:], in0=v[:], in1=mean[:],
                                        op=SUB)
                return sc, v

            with tc.tile_pool(name="bld", bufs=3) as bp, \
                 tc.tile_pool(name="f2p", bufs=1) as f2p, \
                 tc.tile_pool(name="f3p", bufs=2) as f3p, \
                 tc.tile_pool(name="scrp", bufs=1) as scrp, \
                 tc.tile_pool(name="bldps", bufs=2, space="PSUM") as bps, \
                 tc.tile_pool(name="miscps", bufs=1, space="PSUM") as pm, \
                 tc.tile_pool(name="biasps", bufs=2, space="PSUM") as pmb:
                scr = scrp.tile([128, N2 // 8], f32)

                def colstats(name, blocks, P=128):
                    """blocks: list of (ap, width). Returns sum,ssq [P,1]."""
                    nb = len(blocks)
                    sp = cp.tile([P, nb], f32, tag=f"{name}_sp")
                    qp = cp.tile([P, nb], f32, tag=f"{name}_qp")
                    for k, (ap, w) in enumerate(blocks):
                        nc.vector.tensor_reduce(
                            out=sp[:, k:k + 1], in_=ap,
                            axis=mybir.AxisListType.X, op=ADD)
                        nc.scalar.activation(
                            out=scr[:, :w], in_=ap,
                            func=mybir.ActivationFunctionType.Square,
                            accum_out=qp[:, k:k + 1])
                    su = cp.tile([P, 1], f32, tag=f"{name}_su")
                    sq = cp.tile([P, 1], f32, tag=f"{name}_sq")
                    nc.vector.tensor_reduce(out=su[:], in_=sp[:],
                                            axis=mybir.AxisListType.X, op=ADD)
                    nc.vector.tensor_reduce(out=sq[:], in_=qp[:],
                                            axis=mybir.AxisListType.X, op=ADD)
                    return su, sq

                # ---- f2 stats (early, feeds AllReduce) ----
                f2s_t = f2p.tile([128, N2 // 8], f32, tag="f2s")
                nc.scalar.dma_start(out=f2s_t[:], in_=f2ts_d[...])
                sum2, ssq2 = colstats("s2", [(f2s_t[:], N2 // 8)])

                # ---- f3 stats via Gram matmul (early) ----
                gram_ps = pm.tile([65, 65], f32, space="PSUM", tag="gram")
                for ch in range(4):
                    f3c = f3p.tile([128, 60, 65], f32, tag="f3c")
                    nc.scalar.dma_start(
                        out=f3c[:], in_=f3s_d[:, ch * 60:(ch + 1) * 60, :])
                    for m in range(60):
                        k = ch * 60 + m
                        nc.tensor.matmul(out=gram_ps[:], lhsT=f3c[:, m, :],
                                         rhs=f3c[:, m, :],
                                         start=(k == 0), stop=(k == 239))
                gram_sb = cp.tile([65, 65], f32)
                nc.vector.tensor_copy(out=gram_sb[:], in_=gram_ps[:])
                diag3 = cp.tile([65, 65], f32)
                nc.vector.tensor_tensor(out=diag3[:], in0=gram_sb[:],
                                        in1=ident[:65, :65], op=MUL)
                ssq3 = cp.tile([65, 1], f32)
                nc.vector.tensor_reduce(out=ssq3[:], in_=diag3[:],
                                        axis=mybir.AxisListType.X, op=ADD)

                pack = cp.tile([128, 4], f32)
                nc.vector.memset(pack[:], 0.0)
                nc.vector.tensor_copy(out=pack[:, 0:1], in_=sum2[:])
                nc.vector.tensor_copy(out=pack[:, 1:2], in_=ssq2[:])
                nc.vector.tensor_copy(out=pack[:65, 2:3],
                                      in_=gram_sb[:, 64:65])
                nc.vector.tensor_copy(out=pack[:65, 3:4], in_=ssq3[:])
                nc.sync.dma_start(out=cc_in_d[:, :], in_=pack[:])
                nc.gpsimd.collective_compute(
                    "AllReduce", ADD, replica_groups=[list(range(N_CORES))],
                    ins=[cc_in_d[:, :]], outs=[cc_out_d[:, :]])

                # ---- proj table builds (levels 0 and 1), streamed ----
                b_all = cp.tile([1, DOUT], f32)

                def bias_contrib(v, w_t, nch, first):
                    bl = pmb.tile([1, DOUT], f32, space="PSUM", tag="bl")
                    for cc in range(nch):
                        nc.tensor.matmul(
                            out=bl[:], lhsT=v[:, cc:cc + 1],
                            rhs=w_t[:, cc, :] if nch > 1 else w_t[:],
                            start=(cc == 0), stop=(cc == nch - 1))
                    if first:
                        nc.vector.tensor_copy(out=b_all[:], in_=bl[:])
                    else:
                        nc.vector.tensor_tensor(out=b_all[:], in0=b_all[:],
                                                in1=bl[:], op=ADD)

                def build_level(lvl, src_d, w_dram, g_dram, bg_dram, nch, n,
                                blk, proj_dram, first_b):
                    nrt = n // 125
                    rt_per_blk = blk // 125
                    nblk = n // blk
                    w_t = cp.tile([128, nch, DOUT], f32, tag=f"w{lvl}")
                    g_t = cp.tile([128, nch], f32, tag=f"g{lvl}")
                    bg_t = cp.tile([128, nch], f32, tag=f"bg{lvl}")
                    nc.sync.dma_start(out=w_t[:], in_=w_dram[...])
                    nc.sync.dma_start(out=g_t[:], in_=g_dram[...])
                    nc.sync.dma_start(out=bg_t[:], in_=bg_dram[...])
                    # pass A: stats
                    sp_ = cp.tile([128, nch, nblk], f32, tag=f"st{lvl}_sp")
                    qp_ = cp.tile([128, nch, nblk], f32, tag=f"st{lvl}_qp")
                    for b in range(nblk):
                        t = bp.tile([128, nch, blk], f32, tag="blk")
                        nc.scalar.dma_start(
                            out=t[:], in_=src_d[:, :, b * blk:(b + 1) * blk])
                        for cc in range(nch):
                            nc.vector.tensor_reduce(
                                out=sp_[:, cc, b:b + 1], in_=t[:, cc, :],
                                axis=mybir.AxisListType.X, op=ADD)
                            nc.scalar.activation(
                                out=scr[:, :blk], in_=t[:, cc, :],
                                func=mybir.ActivationFunctionType.Square,
                                accum_out=qp_[:, cc, b:b + 1])
                    su = cp.tile([128, nch], f32, tag=f"st{lvl}_su")
                    sq = cp.tile([128, nch], f32, tag=f"st{lvl}_sq")
                    for cc in range(nch):
                        nc.vector.tensor_reduce(
                            out=su[:, cc:cc + 1], in_=sp_[:, cc, :],
                            axis=mybir.AxisListType.X, op=ADD)
                        nc.vector.tensor_reduce(
                            out=sq[:, cc:cc + 1], in_=qp_[:, cc, :],
                            axis=mybir.AxisListType.X, op=ADD)
                    sc, v = fold(su[:], sq[:], g_t[:], bg_t[:], 1.0 / n,
                                 128, nch)
                    for cc in range(nch):
                        nc.vector.tensor_scalar_mul(
                            w_t[:, cc, :], w_t[:, cc, :], sc[:, cc:cc + 1])
                    bias_contrib(v, w_t, nch, first_b)
                    # pass B: proj table
                    for b in range(nblk):
                        t = bp.tile([128, nch, blk], f32, tag="blk")
                        nc.scalar.dma_start(
                            out=t[:], in_=src_d[:, :, b * blk:(b + 1) * blk])
                        for r in range(rt_per_blk):
                            rt = b * rt_per_blk + r
                            ps = bps.tile([125, DOUT], f32, space="PSUM",
                                          tag="pp")
                            for cc in range(nch):
                                nc.tensor.matmul(
                                    out=ps[:],
                                    lhsT=t[:, cc, bass.ts(r, 125)],
                                    rhs=w_t[:, cc, :],
                                    start=(cc == 0), stop=(cc == nch - 1))
                            ob = bp.tile([125, DOUT], f32, tag="ob")
                            nc.vector.tensor_copy(out=ob[:], in_=ps[:])
                            nc.sync.dma_start(
                                out=proj_dram[bass.ts(rt, 125), :], in_=ob[:])

                build_level(0, f0tc_d, w0_d, g0_d, bg0_d, 4, N0, 750,
                            proj0_d, True)
                build_level(1, f1tc_d, w1_d, g1_d, bg1_d, 2, N1, 1500,
                            proj1_d, False)

                # ---- AllReduce readback; fold levels 2 and 3 ----
                gstat = cp.tile([128, 4], f32)
                nc.sync.dma_start(out=gstat[:], in_=cc_out_d[:, :])
                g2_t = cp.tile([128, 1], f32); bg2_t = cp.tile([128, 1], f32)
                nc.sync.dma_start(out=g2_t[:], in_=g2_d[...])
                nc.sync.dma_start(out=bg2_t[:], in_=bg2_d[...])
                sc2, v2 = fold(gstat[:, 0:1], gstat[:, 1:2], g2_t[:],
                               bg2_t[:], 1.0 / N2, 128, 1)
                nc.vector.tensor_scalar_mul(w2_t[:], w2_t[:], sc2[:, 0:1])
                bias_contrib(v2, w2_t, 1, False)

                g3_t = cp.tile([65, 1], f32); bg3_t = cp.tile([65, 1], f32)
                nc.sync.dma_start(out=g3_t[:], in_=g3_d[...])
                nc.sync.dma_start(out=bg3_t[:], in_=bg3_d[...])
                sc3, v3 = fold(gstat[:65, 2:3], gstat[:65, 3:4], g3_t[:],
                               bg3_t[:], 1.0 / N3, 65, 1)
                nc.vector.memset(sc3[64:65, :], 1.0)
                nc.vector.tensor_scalar_mul(w3_t[:], w3_t[:], sc3[:, 0:1])
                bias_contrib(v3, w3_t, 1, False)
                nc.vector.tensor_copy(out=w3_t[64:65, :], in_=b_all[:])

            # ---------- main gather/accumulate loop ----------
            with tc.tile_pool(name="work", bufs=2) as wp, \
                 tc.tile_pool(name="sml", bufs=3) as sp, \
                 tc.tile_pool(name="mps", bufs=2, space="PSUM") as mp:
                for g in range(NG):
                    t0 = wp.tile([128, MCOL, DOUT], f32, tag="t0")
                    t1 = wp.tile([128, MCOL, DOUT], f32, tag="t1")
                    g2p = wp.tile([128, MCOL, 256], f32, tag="g2p")
                    f3g = wp.tile([128, MCOL, 65], f32, tag="f3g")
                    nc.gpsimd.dma_gather(
                        t0[:], proj0_d[:, :], ix_t[0][:, g, :], NIDX, NV[g],
                        DOUT, single_packet=False, queue_num=g % 2)
                    nc.gpsimd.dma_gather(
                        t1[:], proj1_d[:, :], ix_t[1][:, g, :], NIDX, NV[g],
                        DOUT, single_packet=False, queue_num=(g + 1) % 2)
                    nc.gpsimd.dma_gather(
                        g2p[:], f2pair_d[:, :], ix_t[2][:, g, :], NIDX, NV[g],
                        256, single_packet=False, queue_num=g % 2)
                    nc.scalar.dma_start(
                        out=f3g[:], in_=f3s_d[:, g * MCOL:(g + 1) * MCOL, :])
                    # keep even source row, or odd one where parity=1
                    nc.vector.copy_predicated(
                        out=g2p[:, :, :128],
                        mask=par_t[:, g * MCOL:(g + 1) * MCOL, :]
                        .to_broadcast([128, MCOL, 128]),
                        data=g2p[:, :, 128:])
                    for m in range(MCOL):
                        pt2 = mp.tile([128, 128], f32, space="PSUM",
                                      tag="pt2")
                        nc.tensor.transpose(out=pt2[:], in_=g2p[:, m, :128],
                                            identity=ident[:])
                        s2 = sp.tile([128, 128], f32, tag="s2")
                        nc.vector.tensor_copy(out=s2[:], in_=pt2[:])
                        pt3 = mp.tile([65, 128], f32, space="PSUM",
                                      tag="pt3")
                        nc.tensor.transpose(out=pt3[:], in_=f3g[:, m, :],
                                            identity=ident[:])
                        s3 = sp.tile([65, 128], f32, tag="s3")
                        nc.vector.tensor_copy(out=s3[:], in_=pt3[:])
                        acc = mp.tile([128, DOUT], f32, space="PSUM",
                                      tag="acc")
                        nc.tensor.matmul(out=acc[:], lhsT=s2[:], rhs=w2_t[:],
                                         start=True, stop=False)
                        nc.tensor.matmul(out=acc[:], lhsT=s3[:], rhs=w3_t[:],
                                         start=False, stop=True)
                        nc.vector.tensor_tensor(
                            out=t0[:, m, :], in0=t0[:, m, :], in1=t1[:, m, :],
                            op=ADD)
                        nc.vector.tensor_tensor(
                            out=t0[:, m, :], in0=t0[:, m, :], in1=acc[:],
                            op=ADD)
                    nc.sync.dma_start(
                        out=out_d[:, g * MCOL:(g + 1) * MCOL, :], in_=t0[:])
    nc.compile()
    return nc


def _wrap_idx(v):
    """[NS] ints -> [128, NG, NIDX//16] int16: wrap 16, replicate 8x."""
    p = np.full(NPAD, -1, np.int16)
    p[:NS] = v.astype(np.int16)
    w = p.reshape(NG, NIDX // 16, 16).transpose(2, 0, 1)   # [16, NG, 192]
    return np.ascontiguousarray(np.tile(w, (8, 1, 1)))


def _grid(v, tail):
    """[NS, ...] -> [128, NCOL, ...]: j = g*3072 + m*128 + p -> [p, g*24+m]."""
    p = np.zeros((NPAD,) + tail, v.dtype)
    p[:NS] = v
    return np.ascontiguousarray(
        p.reshape(NG, MCOL, 128, *tail).transpose(2, 0, 1, 3)
        .reshape(128, NCOL, *tail))


def _chunked_T(f, nchunk):
    """[N, C] -> [128, nchunk, N]: [p, cc, r] = f[r, cc*128+p]."""
    n = f.shape[0]
    return np.ascontiguousarray(
        f.T.reshape(nchunk, 128, n).transpose(1, 0, 2))


def _chunk_vec(v, nchunk):
    return np.ascontiguousarray(v.reshape(nchunk, 128).T)


def kernel(f0, f1, f2, f3,
           gamma0, beta0, W0, gamma1, beta1, W1,
           gamma2, beta2, W2, gamma3, beta3, W3,
           idx0, idx1, idx2):
    global _prog_cache
    from concourse.bass_utils import run_bass_kernel_spmd

    f0 = np.asarray(f0, np.float32); f1 = np.asarray(f1, np.float32)
    f2 = np.asarray(f2, np.float32); f3 = np.asarray(f3, np.float32)
    idx0 = np.asarray(idx0); idx1 = np.asarray(idx1); idx2 = np.asarray(idx2)
    ga = [np.asarray(g, np.float32) for g in (gamma0, gamma1, gamma2, gamma3)]
    be = [np.asarray(b, np.float32) for b in (beta0, beta1, beta2, beta3)]

    common = {
        "f0tc": _chunked_T(f0, 4),
        "f1tc": _chunked_T(f1, 2),
        "f2pair": np.ascontiguousarray(f2.reshape(N2 // 2, 256)),
        "w0tc": _chunked_T(np.asarray(W0, np.float32), 4),
        "w1tc": _chunked_T(np.asarray(W1, np.float32), 2),
        "w2t": np.ascontiguousarray(np.asarray(W2, np.float32).T),
        "w3te": np.concatenate(
            [np.asarray(W3, np.float32).T, np.zeros((1, DOUT), np.float32)]),
        "g0c": _chunk_vec(ga[0], 4), "bg0c": _chunk_vec(be[0] / ga[0], 4),
        "g1c": _chunk_vec(ga[1], 2), "bg1c": _chunk_vec(be[1] / ga[1], 2),
        "g2c": ga[2][:, None].copy(), "bg2c": (be[2] / ga[2])[:, None].copy(),
        "g3ce": np.concatenate([ga[3], [1.0]]).astype(np.float32)[:, None],
        "bg3ce": np.concatenate(
            [be[3] / ga[3], [0.0]]).astype(np.float32)[:, None],
    }
    in_maps = []
    for c in range(N_CORES):
        s = slice(c * NS, (c + 1) * NS)
        i2 = np.asarray(idx2[s])
        f3e = np.concatenate([f3[s], np.ones((NS, 1), np.float32)], axis=1)
        m = dict(common)
        m["f2ts"] = np.ascontiguousarray(
            f2[c * (N2 // 8):(c + 1) * (N2 // 8)].T)
        m["f3s"] = _grid(f3e, (65,))
        m["idx0w"] = _wrap_idx(np.asarray(idx0[s]))
        m["idx1w"] = _wrap_idx(np.asarray(idx1[s]))
        m["idx2w"] = _wrap_idx(i2 >> 1)
        m["par2"] = _grid((i2 & 1).astype(np.int8)[:, None], (1,))
        in_maps.append(m)

    global _last_in_maps
    _last_in_maps = in_maps
    if _prog_cache is None:
        _prog_cache = _build_program()
    res = run_bass_kernel_spmd(_prog_cache, in_maps,
                               core_ids=list(range(N_CORES)))

    parts = []
    for c in range(N_CORES):
        o = res.results[c]["out"]                    # [128, NCOL, 256]
        o = o.reshape(128, NG, MCOL, DOUT).transpose(1, 2, 0, 3)
        parts.append(o.reshape(NPAD, DOUT)[:NS])
    return np.concatenate(parts, axis=0)


# revision 13
# speedup vs baseline: 1.4138x; 1.4138x over previous
"""Trainium2 Bass kernel for nn_Decoder_3461743640648 (gnn_message_passing).

acc = bn_linear(f0)[idx0] + bn_linear(f1)[idx1] + bn_linear(f2)[idx2]
      + bn_linear(f3)                                  -> [240000, 256]

Strategy (8 cores, data-parallel over the 240000 fine points):
 - Training-mode BN folded into the linear: proj_l = f_l @ (W_l*scale_l)^T
   + b'_l with scale_l = gamma_l*rsqrt(var_l+eps) and
   b'_l = ((beta_l/gamma_l)*std_l - mean_l) @ (W_l*scale_l)^T. All four b'_l
   are summed into one b_all added once via a ones-column in the f3 operand.
 - proj0 [3750,256] / proj1 [15000,256] tables built redundantly on every
   core (from host-pre-transposed chunk layouts; streamed in row blocks,
   one stats pass + one matmul pass), stored to local DRAM, then
   row-gathered at fine resolution with dma_gather (int16 indices,
   3072 rows/call, multi-packet, alternating SWDGE queues).
 - Level 2 gathers RAW f2 rows as pairs: f2 viewed [30000, 256] so idx2>>1
   fits int16; a DVE predicated-copy selects the odd row where parity=1.
   The 128->256 projection happens on the PE at fine resolution.
 - f0/f1 stats: free-axis reduce + ACT Square(accum_out) on transposed
   blocks. f2 stats from a host-transposed per-core row-shard. f3 stats
   from one accumulated Gram matmul (the ones column makes Gram[:,64] the
   per-channel sums; the diagonal is sumsq). f2/f3 partial stats cross
   cores via one 2KB AllReduce, overlapped with the proj table builds.
 - Per-core point layout is column-major within gather calls:
   j = g*3072 + m*128 + p  ->  out[p, g*24+m, :].
"""
import sys

sys.path.insert(0, "/opt/trn_rl_repo")

import numpy as np  # noqa: E402

EPS = 1e-5
N_CORES = 8
NF = 240000          # fine points
NS = NF // N_CORES   # 30000 per core
NIDX = 2048          # rows per dma_gather call
NG = 15              # gather calls (groups) per core
NPAD = NG * NIDX     # 30720
MCOL = NIDX // 128   # 24 sub-tile columns per group
NCOL = NG * MCOL     # 240 output columns
N0, N1, N2, N3 = 3750, 15000, 60000, 240000
DOUT = 256

_prog_cache = None
_last_in_maps = None


def _build_program():
    from concourse import bass, bacc, mybir, tile
    from concourse.library_config import mlp
    from concourse.masks import make_identity

    f32 = mybir.dt.float32
    f32r = mybir.dt.float32r
    i16 = mybir.dt.int16
    ADD = mybir.AluOpType.add
    MUL = mybir.AluOpType.mult
    SUB = mybir.AluOpType.subtract

    nc = bacc.Bacc("TRN2", target_bir_lowering=False, debug=False,
                   num_devices=N_CORES, num_swdge_queues=2)

    def din(name, shape, dt=f32):
        return nc.dram_tensor(name, shape, dt, kind="ExternalInput").ap()

    # replicated inputs
    f0tc_d = din("f0tc", [128, 4, N0], f32r)  # f0.T chunk layout
    f1tc_d = din("f1tc", [128, 2, N1], f32r)  # f1.T chunk layout
    f2pair_d = din("f2pair", [N2 // 2, 256])  # f2 viewed as row pairs
    w0_d = din("w0tc", [128, 4, DOUT], f32r)
    w1_d = din("w1tc", [128, 2, DOUT], f32r)
    w2_d = din("w2t", [128, DOUT], f32r)
    w3_d = din("w3te", [65, DOUT], f32r)      # W3.T + zeros row
    g0_d = din("g0c", [128, 4]); bg0_d = din("bg0c", [128, 4])
    g1_d = din("g1c", [128, 2]); bg1_d = din("bg1c", [128, 2])
    g2_d = din("g2c", [128, 1]); bg2_d = din("bg2c", [128, 1])
    g3_d = din("g3ce", [65, 1]); bg3_d = din("bg3ce", [65, 1])
    # per-core inputs
    f2ts_d = din("f2ts", [128, N2 // 8])      # transposed f2 row-shard
    f3ts_d = din("f3ts", [65, NPAD], f32r)    # (f3 shard + ones col)^T, j-order
    ix0_d = din("idx0w", [128, NG, NIDX // 16], i16)
    ix1_d = din("idx1w", [128, NG, NIDX // 16], i16)
    ix2_d = din("idx2w", [128, NG, NIDX // 16], i16)
    par_d = din("par2", [128, NCOL, 1], mybir.dt.int8)
    out_d = nc.dram_tensor("out", [128, NCOL, DOUT], f32,
                           kind="ExternalOutput").ap()
    # internal DRAM
    proj0_d = nc.dram_tensor("proj0", [N0, DOUT], f32).ap()
    proj1_d = nc.dram_tensor("proj1", [N1, DOUT], f32).ap()
    cc_in_d = nc.dram_tensor("cc_in", [128, 4], f32).ap()
    cc_out_d = nc.dram_tensor("cc_out", [128, 4], f32,
                              addr_space="Shared").ap()

    NV = [NIDX] * (NG - 1) + [NS - (NG - 1) * NIDX]  # valid idx per call

    with tile.TileContext(nc) as tc:
        with tc.tile_pool(name="const", bufs=1) as cp:
            nc.gpsimd.load_library(mlp)
            ident = cp.tile([128, 128], f32)
            make_identity(nc, ident[:])
            eps_t = cp.tile([128, 1], f32)
            nc.vector.memset(eps_t[:], EPS)

            ix_t = []
            for ix_d in (ix0_d, ix1_d, ix2_d):
                t = cp.tile([128, NG, NIDX // 16], i16, tag=f"ix{len(ix_t)}")
                nc.sync.dma_start(out=t[:], in_=ix_d[...])
                ix_t.append(t)
            par_t = cp.tile([128, NCOL, 1], mybir.dt.int8)
            nc.sync.dma_start(out=par_t[:], in_=par_d[...])

            w2_t = cp.tile([128, DOUT], f32r)
            w3_t = cp.tile([65, DOUT], f32r)
            w2s = cp.tile([128, DOUT], f32r, tag="w2s")
            w3s = cp.tile([65, DOUT], f32r, tag="w3s")
            nc.sync.dma_start(out=w2_t[:], in_=w2_d[...])
            nc.sync.dma_start(out=w3_t[:], in_=w3_d[...])

            def fold(sum_ap, ssq_ap, g_ap, bg_ap, inv_n, P, K):
                """-> (scale [P,K], v [P,K]); v @ (W*scale)^T = bias b'."""
                mean = cp.tile([P, K], f32, tag="fold_mean")
                var = cp.tile([P, K], f32, tag="fold_var")
                std = cp.tile([P, K], f32, tag="fold_std")
                rstd = cp.tile([P, K], f32, tag="fold_rstd")
                sc = cp.tile([P, K], f32, tag="fold_sc")
                v = cp.tile([P, K], f32, tag="fold_v")
                nc.vector.tensor_scalar_mul(mean[:], sum_ap, inv_n)
                nc.vector.tensor_scalar_mul(var[:], ssq_ap, inv_n)
                nc.vector.tensor_tensor(out=std[:], in0=mean[:], in1=mean[:],
                                        op=MUL)
                nc.vector.tensor_tensor(out=var[:], in0=var[:], in1=std[:],
                                        op=SUB)
                nc.scalar.activation(
                    out=std[:], in_=var[:],
                    func=mybir.ActivationFunctionType.Sqrt,
                    bias=eps_t[:P, :])
                nc.vector.reciprocal(out=rstd[:], in_=std[:])
                nc.vector.tensor_tensor(out=sc[:], in0=rstd[:], in1=g_ap,
                                        op=MUL)
                nc.vector.tensor_tensor(out=v[:], in0=std[:], in1=bg_ap,
                                        op=MUL)
                nc.vector.tensor_tensor(out=v[:], in0=v[:], in1=mean[:],
                                        op=SUB)
                return sc, v

            with tc.tile_pool(name="bld", bufs=3) as bp, \
                 tc.tile_pool(name="f2p", bufs=1) as f2p, \
                 tc.tile_pool(name="f3p", bufs=2) as f3p, \
                 tc.tile_pool(name="scrp", bufs=1) as scrp, \
                 tc.tile_pool(name="bldps", bufs=2, space="PSUM") as bps, \
                 tc.tile_pool(name="miscps", bufs=1, space="PSUM") as pm, \
                 tc.tile_pool(name="biasps", bufs=2, space="PSUM") as pmb:
                scr = scrp.tile([128, N2 // 8], f32)

                def colstats(name, blocks, P=128):
                    """blocks: list of (ap, width). Returns sum,ssq [P,1]."""
                    nb = len(blocks)
                    sp = cp.tile([P, nb], f32, tag=f"{name}_sp")
                    qp = cp.tile([P, nb], f32, tag=f"{name}_qp")
                    for k, (ap, w) in enumerate(blocks):
                        nc.vector.tensor_reduce(
                            out=sp[:, k:k + 1], in_=ap,
                            axis=mybir.AxisListType.X, op=ADD)
                        nc.scalar.activation(
                            out=scr[:, :w], in_=ap,
                            func=mybir.ActivationFunctionType.Square,
                            accum_out=qp[:, k:k + 1])
                    su = cp.tile([P, 1], f32, tag=f"{name}_su")
                    sq = cp.tile([P, 1], f32, tag=f"{name}_sq")
                    nc.vector.tensor_reduce(out=su[:], in_=sp[:],
                                            axis=mybir.AxisListType.X, op=ADD)
                    nc.vector.tensor_reduce(out=sq[:], in_=qp[:],
                                            axis=mybir.AxisListType.X, op=ADD)
                    return su, sq

                # ---- f2 stats (early, feeds AllReduce) ----
                f2s_t = f2p.tile([128, N2 // 8], f32, tag="f2s")
                nc.scalar.dma_start(out=f2s_t[:], in_=f2ts_d[...])
                sum2, ssq2 = colstats("s2", [(f2s_t[:], N2 // 8)])

                # ---- f3 stats from the transposed layout (early) ----
                sp3 = cp.tile([65, NG], f32, tag="sp3")
                qp3 = cp.tile([65, NG], f32, tag="qp3")
                for g in range(NG):
                    f3c = f3p.tile([65, NIDX], f32r, tag="f3c")
                    nc.scalar.dma_start(
                        out=f3c[:],
                        in_=f3ts_d[:, g * NIDX:(g + 1) * NIDX])
                    f3cf = f3c[:].bitcast(f32)
                    nc.vector.tensor_reduce(
                        out=sp3[:, g:g + 1], in_=f3cf,
                        axis=mybir.AxisListType.X, op=ADD)
                    nc.scalar.activation(
                        out=scr[:65, :NIDX], in_=f3cf,
                        func=mybir.ActivationFunctionType.Square,
                        accum_out=qp3[:, g:g + 1])
                sum3 = cp.tile([65, 1], f32, tag="sum3")
                ssq3 = cp.tile([65, 1], f32, tag="ssq3")
                nc.vector.tensor_reduce(out=sum3[:], in_=sp3[:],
                                        axis=mybir.AxisListType.X, op=ADD)
                nc.vector.tensor_reduce(out=ssq3[:], in_=qp3[:],
                                        axis=mybir.AxisListType.X, op=ADD)

                pack = cp.tile([128, 4], f32)
                nc.vector.memset(pack[:], 0.0)
                nc.vector.tensor_copy(out=pack[:, 0:1], in_=sum2[:])
                nc.vector.tensor_copy(out=pack[:, 1:2], in_=ssq2[:])
                nc.vector.tensor_copy(out=pack[:65, 2:3], in_=sum3[:])
                nc.vector.tensor_copy(out=pack[:65, 3:4], in_=ssq3[:])
                nc.sync.dma_start(out=cc_in_d[:, :], in_=pack[:])
                nc.gpsimd.collective_compute(
                    "AllReduce", ADD, replica_groups=[list(range(N_CORES))],
                    ins=[cc_in_d[:, :]], outs=[cc_out_d[:, :]])

                # ---- proj table builds (levels 0 and 1), streamed ----
                b_all = cp.tile([1, DOUT], f32)

                def bias_contrib(v, w_t, nch, first):
                    P = v.shape[0]
                    v_r = cp.tile([P, nch], f32r, tag="v_r")
                    nc.vector.tensor_copy(out=v_r[:], in_=v[:])
                    bl = pmb.tile([1, DOUT], f32, space="PSUM", tag="bl")
                    for cc in range(nch):
                        nc.tensor.matmul(
                            out=bl[:], lhsT=v_r[:, cc:cc + 1],
                            rhs=w_t[:, cc, :] if nch > 1 else w_t[:],
                            start=(cc == 0), stop=(cc == nch - 1))
                    if first:
                        nc.vector.tensor_copy(out=b_all[:], in_=bl[:])
                    else:
                        nc.vector.tensor_tensor(out=b_all[:], in0=b_all[:],
                                                in1=bl[:], op=ADD)

                def build_level(lvl, src_d, w_dram, g_dram, bg_dram, nch, n,
                                blk, proj_dram, first_b):
                    nrt = n // 125
                    rt_per_blk = blk // 125
                    nblk = n // blk
                    w_t = cp.tile([128, nch, DOUT], f32r, tag=f"w{lvl}")
                    g_t = cp.tile([128, nch], f32, tag=f"g{lvl}")
                    bg_t = cp.tile([128, nch], f32, tag=f"bg{lvl}")
                    nc.sync.dma_start(out=w_t[:], in_=w_dram[...])
                    nc.sync.dma_start(out=g_t[:], in_=g_dram[...])
                    nc.sync.dma_start(out=bg_t[:], in_=bg_dram[...])
                    # pass A: stats
                    sp_ = cp.tile([128, nch, nblk], f32, tag=f"st{lvl}_sp")
                    qp_ = cp.tile([128, nch, nblk], f32, tag=f"st{lvl}_qp")
                    for b in range(nblk):
                        t = bp.tile([128, nch, blk], f32r, tag="blk")
                        nc.scalar.dma_start(
                            out=t[:], in_=src_d[:, :, b * blk:(b + 1) * blk])
                        for cc in range(nch):
                            tf = t[:, cc, :].bitcast(f32)
                            nc.vector.tensor_reduce(
                                out=sp_[:, cc, b:b + 1], in_=tf,
                                axis=mybir.AxisListType.X, op=ADD)
                            nc.scalar.activation(
                                out=scr[:, :blk], in_=tf,
                                func=mybir.ActivationFunctionType.Square,
                                accum_out=qp_[:, cc, b:b + 1])
                    su = cp.tile([128, nch], f32, tag=f"st{lvl}_su")
                    sq = cp.tile([128, nch], f32, tag=f"st{lvl}_sq")
                    for cc in range(nch):
                        nc.vector.tensor_reduce(
                            out=su[:, cc:cc + 1], in_=sp_[:, cc, :],
                            axis=mybir.AxisListType.X, op=ADD)
                        nc.vector.tensor_reduce(
                            out=sq[:, cc:cc + 1], in_=qp_[:, cc, :],
                            axis=mybir.AxisListType.X, op=ADD)
                    sc, v = fold(su[:], sq[:], g_t[:], bg_t[:], 1.0 / n,
                                 128, nch)
                    for cc in range(nch):
                        nc.vector.tensor_scalar_mul(
                            w_t[:, cc, :], w_t[:, cc, :].bitcast(f32),
                            sc[:, cc:cc + 1])
                    bias_contrib(v, w_t, nch, first_b)
                    # pass B: proj table
                    for b in range(nblk):
                        t = bp.tile([128, nch, blk], f32r, tag="blk")
                        nc.scalar.dma_start(
                            out=t[:], in_=src_d[:, :, b * blk:(b + 1) * blk])
                        for r in range(rt_per_blk):
                            rt = b * rt_per_blk + r
                            ps = bps.tile([125, DOUT], f32, space="PSUM",
                                          tag="pp")
                            for cc in range(nch):
                                nc.tensor.matmul(
                                    out=ps[:],
                                    lhsT=t[:, cc, bass.ts(r, 125)],
                                    rhs=w_t[:, cc, :],
                                    start=(cc == 0), stop=(cc == nch - 1))
                            ob = bp.tile([125, DOUT], f32, tag="ob")
                            nc.vector.tensor_copy(out=ob[:], in_=ps[:])
                            nc.sync.dma_start(
                                out=proj_dram[bass.ts(rt, 125), :], in_=ob[:])

                build_level(0, f0tc_d, w0_d, g0_d, bg0_d, 4, N0, 750,
                            proj0_d, True)
                build_level(1, f1tc_d, w1_d, g1_d, bg1_d, 2, N1, 1500,
                            proj1_d, False)

                # ---- AllReduce readback; fold levels 2 and 3 ----
                gstat = cp.tile([128, 4], f32)
                nc.sync.dma_start(out=gstat[:], in_=cc_out_d[:, :])
                g2_t = cp.tile([128, 1], f32); bg2_t = cp.tile([128, 1], f32)
                nc.sync.dma_start(out=g2_t[:], in_=g2_d[...])
                nc.sync.dma_start(out=bg2_t[:], in_=bg2_d[...])
                sc2, v2 = fold(gstat[:, 0:1], gstat[:, 1:2], g2_t[:],
                               bg2_t[:], 1.0 / N2, 128, 1)
                nc.vector.tensor_scalar_mul(w2s[:], w2_t[:].bitcast(f32),
                                            sc2[:, 0:1])
                bias_contrib(v2, w2s, 1, False)

                g3_t = cp.tile([65, 1], f32); bg3_t = cp.tile([65, 1], f32)
                nc.sync.dma_start(out=g3_t[:], in_=g3_d[...])
                nc.sync.dma_start(out=bg3_t[:], in_=bg3_d[...])
                sc3, v3 = fold(gstat[:65, 2:3], gstat[:65, 3:4], g3_t[:],
                               bg3_t[:], 1.0 / N3, 65, 1)
                nc.vector.memset(sc3[64:65, :], 1.0)
                nc.vector.tensor_scalar_mul(w3s[:], w3_t[:].bitcast(f32),
                                            sc3[:, 0:1])
                bias_contrib(v3, w3s, 1, False)
                nc.vector.tensor_copy(out=w3s[64:65, :], in_=b_all[:])

            # ---------- main gather/accumulate loop ----------
            with tc.tile_pool(name="work", bufs=2) as wp, \
                 tc.tile_pool(name="f3m", bufs=2) as f3m, \
                 tc.tile_pool(name="sml", bufs=3) as sp, \
                 tc.tile_pool(name="mps", bufs=2, space="PSUM") as mp:
                for g in range(NG):
                    t0 = wp.tile([128, MCOL, DOUT], f32, tag="t0")
                    t1 = wp.tile([128, MCOL, DOUT], f32, tag="t1")
                    g2p = wp.tile([128, MCOL, 256], f32, tag="g2p")
                    f3tg = f3m.tile([65, NIDX], f32r, tag="f3tg")
                    nc.gpsimd.dma_gather(
                        t0[:], proj0_d[:, :], ix_t[0][:, g, :], NIDX, NV[g],
                        DOUT, single_packet=False, queue_num=g % 2)
                    nc.gpsimd.dma_gather(
                        t1[:], proj1_d[:, :], ix_t[1][:, g, :], NIDX, NV[g],
                        DOUT, single_packet=False, queue_num=(g + 1) % 2)
                    nc.gpsimd.dma_gather(
                        g2p[:], f2pair_d[:, :], ix_t[2][:, g, :], NIDX, NV[g],
                        256, single_packet=False, queue_num=g % 2)
                    nc.scalar.dma_start(
                        out=f3tg[:],
                        in_=f3ts_d[:, g * NIDX:(g + 1) * NIDX])
                    # keep even source row, or odd one where parity=1
                    nc.vector.copy_predicated(
                        out=g2p[:, :, :128],
                        mask=par_t[:, g * MCOL:(g + 1) * MCOL, :]
                        .to_broadcast([128, MCOL, 128]),
                        data=g2p[:, :, 128:])
                    for m in range(MCOL):
                        pt2 = mp.tile([128, 128], f32, space="PSUM",
                                      tag="pt2")
                        nc.tensor.transpose(out=pt2[:], in_=g2p[:, m, :128],
                                            identity=ident[:])
                        s2 = sp.tile([128, 128], f32r, tag="s2")
                        nc.vector.tensor_copy(out=s2[:], in_=pt2[:])
                        acc = mp.tile([128, DOUT], f32, space="PSUM",
                                      tag="acc")
                        nc.tensor.matmul(out=acc[:], lhsT=s2[:], rhs=w2s[:],
                                         start=True, stop=False)
                        nc.tensor.matmul(
                            out=acc[:],
                            lhsT=f3tg[:, m * 128:(m + 1) * 128],
                            rhs=w3s[:], start=False, stop=True)
                        nc.vector.tensor_tensor(
                            out=t0[:, m, :], in0=t0[:, m, :], in1=t1[:, m, :],
                            op=ADD)
                        nc.vector.tensor_tensor(
                            out=t0[:, m, :], in0=t0[:, m, :], in1=acc[:],
                            op=ADD)
                    nc.sync.dma_start(
                        out=out_d[:, g * MCOL:(g + 1) * MCOL, :], in_=t0[:])
    nc.compile()
    return nc


def _wrap_idx(v):
    """[NS] ints -> [128, NG, NIDX//16] int16: wrap 16, replicate 8x."""
    p = np.full(NPAD, -1, np.int16)
    p[:NS] = v.astype(np.int16)
    w = p.reshape(NG, NIDX // 16, 16).transpose(2, 0, 1)   # [16, NG, 192]
    return np.ascontiguousarray(np.tile(w, (8, 1, 1)))


def _grid(v, tail):
    """[NS, ...] -> [128, NCOL, ...]: j = g*3072 + m*128 + p -> [p, g*24+m]."""
    p = np.zeros((NPAD,) + tail, v.dtype)
    p[:NS] = v
    return np.ascontiguousarray(
        p.reshape(NG, MCOL, 128, *tail).transpose(2, 0, 1, 3)
        .reshape(128, NCOL, *tail))


def _chunked_T(f, nchunk):
    """[N, C] -> [128, nchunk, N]: [p, cc, r] = f[r, cc*128+p]."""
    n = f.shape[0]
    return np.ascontiguousarray(
        f.T.reshape(nchunk, 128, n).transpose(1, 0, 2))


def _chunk_vec(v, nchunk):
    return np.ascontiguousarray(v.reshape(nchunk, 128).T)


def kernel(f0, f1, f2, f3,
           gamma0, beta0, W0, gamma1, beta1, W1,
           gamma2, beta2, W2, gamma3, beta3, W3,
           idx0, idx1, idx2):
    global _prog_cache
    from concourse.bass_utils import run_bass_kernel_spmd

    f0 = np.asarray(f0, np.float32); f1 = np.asarray(f1, np.float32)
    f2 = np.asarray(f2, np.float32); f3 = np.asarray(f3, np.float32)
    idx0 = np.asarray(idx0); idx1 = np.asarray(idx1); idx2 = np.asarray(idx2)
    ga = [np.asarray(g, np.float32) for g in (gamma0, gamma1, gamma2, gamma3)]
    be = [np.asarray(b, np.float32) for b in (beta0, beta1, beta2, beta3)]

    common = {
        "f0tc": _chunked_T(f0, 4),
        "f1tc": _chunked_T(f1, 2),
        "f2pair": np.ascontiguousarray(f2.reshape(N2 // 2, 256)),
        "w0tc": _chunked_T(np.asarray(W0, np.float32), 4),
        "w1tc": _chunked_T(np.asarray(W1, np.float32), 2),
        "w2t": np.ascontiguousarray(np.asarray(W2, np.float32).T),
        "w3te": np.concatenate(
            [np.asarray(W3, np.float32).T, np.zeros((1, DOUT), np.float32)]),
        "g0c": _chunk_vec(ga[0], 4), "bg0c": _chunk_vec(be[0] / ga[0], 4),
        "g1c": _chunk_vec(ga[1], 2), "bg1c": _chunk_vec(be[1] / ga[1], 2),
        "g2c": ga[2][:, None].copy(), "bg2c": (be[2] / ga[2])[:, None].copy(),
        "g3ce": np.concatenate([ga[3], [1.0]]).astype(np.float32)[:, None],
        "bg3ce": np.concatenate(
            [be[3] / ga[3], [0.0]]).astype(np.float32)[:, None],
    }
    in_maps = []
    for c in range(N_CORES):
        s = slice(c * NS, (c + 1) * NS)
        i2 = np.asarray(idx2[s])
        f3e = np.concatenate([f3[s], np.ones((NS, 1), np.float32)], axis=1)
        f3pad = np.zeros((NPAD, 65), np.float32)
        f3pad[:NS] = f3e
        m = dict(common)
        m["f2ts"] = np.ascontiguousarray(
            f2[c * (N2 // 8):(c + 1) * (N2 // 8)].T)
        m["f3ts"] = np.ascontiguousarray(f3pad.T)
        m["idx0w"] = _wrap_idx(np.asarray(idx0[s]))
        m["idx1w"] = _wrap_idx(np.asarray(idx1[s]))
        m["idx2w"] = _wrap_idx(i2 >> 1)
        m["par2"] = _grid((i2 & 1).astype(np.int8)[:, None], (1,))
        in_maps.append(m)

    global _last_in_maps
    _last_in_maps = in_maps
    if _prog_cache is None:
        _prog_cache = _build_program()
    res = run_bass_kernel_spmd(_prog_cache, in_maps,
                               core_ids=list(range(N_CORES)))

    parts = []
    for c in range(N_CORES):
        o = res.results[c]["out"]                    # [128, NCOL, 256]
        o = o.reshape(128, NG, MCOL, DOUT).transpose(1, 2, 0, 3)
        parts.append(o.reshape(NPAD, DOUT)[:NS])
    return np.concatenate(parts, axis=0)
